# revision 2
# baseline (speedup 1.0000x reference)
"""Trainium2 Bass kernel for nn_Conditionals (DAG-MLP scan).

Strategy
--------
Shard the D=32 dags across 8 cores (4 dags/core). Per dag, keep the
scan state transposed in SBUF as S[p, b] (node-slot on partitions,
batch on the free dim), stored *in topological-order coordinates*:
physical row p holds the value of node order[d][p]. In these
coordinates every step t writes row t, so the instruction stream is
identical across cores (SPMD) — all data dependence is folded into
host-precomputed per-step weights.

Per step t (node n = order[d][t]) the reference computes
    h   = leaky_relu(concat(outputs * mask, x[:, n]) @ W1[n].T + b1[n])
    out = h @ W2[n] + b2[n]
With the state seeded as S[p] = x[:, order[p]] (value of an
uncomputed node IS its x column, consumed via W1's x-column weight)
this collapses to one K=101 matmul per step:
    Wf[p, h] = A[od[p], od[t]] * (p < t) * W1[n][h, od[p]]   (p < 100)
    Wf[t, h] = W1[n][h, 100]          (x column; row t still holds x)
    Wf[100, h] = b1[n]                (constant-ones state row 100)
    h = leaky_relu(Wf.T @ S)
The do_idx intervention step writes back x[:, do_idx], reproduced
exactly from the seed row through the two-channel identity
    x = c0*lrelu(x) + c1*lrelu(-x),  c0 = -c1 = 0.99/0.9999,
keeping the program uniform across dags/cores.

Device layout per core: one state tensor Sall[101, 4*8192] (4 dags
side by side), per-step folded weights streamed from DRAM, layer-1
into PSUM [128, 512] (4 dags x H=32 stacked via PE column tiling),
LeakyReLU on the scalar engine, layer-2 with a block-diagonal W2
[128, 4], PSUM->SBUF copy (+b2) on the vector engine, and one DMA per
half-step scattering the 4 dag rows back into Sall row t.
"""

import os
import sys

sys.path.insert(0, "/opt/trn_rl_repo")

import numpy as np

B = 8192
N = 100
D = 32
H = 32
NCORES = 8
DPC = D // NCORES  # dags per core
CW = 512           # matmul chunk width (one PSUM bank of fp32)
CH = B // CW       # chunks per dag
NP1 = N + 1        # state rows incl. constant-ones row

C0 = np.float32(0.99 / 0.9999)  # lrelu(x), lrelu(-x) -> x reconstruction


def _pack_core(core, x, A, order, do_idx, W1, b1, W2, b2):
    """Host-side fold of masks/params into per-step arrays for one core."""
    xT = np.ascontiguousarray(x.T.astype(np.float32))  # [N, B]
    WD = np.zeros((N, NP1, 4 * H), np.float32)    # [t, p, 32d+h]
    W2D = np.zeros((4 * H, 4 * N), np.float32)    # [32d+h, 4t+d]
    B2D = np.zeros((DPC, N), np.float32)          # [d, t]
    XT = np.empty((DPC, NP1, B), np.float32)      # [d, p(+ones), b]
    tri = np.tril(np.ones((N, N), np.float32), k=-1)  # [t, p] strict lower
    idx = np.arange(N)
    for d in range(DPC):
        dag = core * DPC + d
        od = order[dag].astype(np.int64)
        Aperm = A[dag][np.ix_(od, od)]            # [p, t]
        W1p = W1[od]                              # [t, H, N+1]
        W1sel = W1p[:, :, :N][:, :, od]           # [t, h, p]
        M = Aperm.T * tri                         # [t, p]
        Wf = np.einsum("tp,thp->tph", M, W1sel)   # [t, p, h]
        Wf[idx, idx, :] = W1p[:, :, N]            # x-column weight on diag
        full = np.zeros((N, NP1, H), np.float32)
        full[:, :N, :] = Wf
        full[:, N, :] = b1[od]                    # ones-row -> +b1
        W2v = W2[od].astype(np.float32).copy()    # [t, H]
        b2v = b2[od].astype(np.float32).copy()    # [t]
        t0 = int(np.where(od == do_idx)[0][0])    # the intervention step
        full[t0] = 0.0
        full[t0, t0, 0] = 1.0
        full[t0, t0, 1] = -1.0
        W2v[t0] = 0.0
        W2v[t0, 0] = C0
        W2v[t0, 1] = -C0
        b2v[t0] = 0.0
        WD[:, :, 32 * d : 32 * (d + 1)] = full
        W2D[32 * d : 32 * (d + 1), 4 * idx + d] = W2v.T
        B2D[d] = b2v
        XT[d, :N, :] = xT[od]
        XT[d, N, :] = 1.0
    return {
        "XT": np.ascontiguousarray(XT.reshape(DPC * NP1, B)),
        "WD": WD,
        "W2D": W2D,
        "B2D": B2D,
    }


def _emulate_core(m):
    """Numpy emulation of the device program (packing validation)."""
    S = m["XT"].reshape(DPC, NP1, B).transpose(1, 0, 2).reshape(NP1, DPC * B).copy()
    # device Sall layout: [101, 4*B] with dag-major free dim
    S = np.empty((NP1, DPC * B), np.float32)
    XT = m["XT"].reshape(DPC, NP1, B)
    for d in range(DPC):
        S[:, d * B : (d + 1) * B] = XT[d]
    for t in range(N):
        wt = m["WD"][t]                           # [101, 128]
        for d in range(DPC):
            rhs = S[:, d * B : (d + 1) * B]       # [101, B]
            pre = wt[:, 32 * d : 32 * (d + 1)].T @ rhs  # [32, B]
            h = np.where(pre >= 0, pre, np.float32(0.01) * pre)
            w2col = m["W2D"][:, 4 * t + d][32 * d : 32 * (d + 1)]
            out2 = w2col @ h + m["B2D"][d, t]
            S[t, d * B : (d + 1) * B] = out2
    return S[:N]  # [100, 4*B]


_PROGRAM_CACHE = {}


def _build_program(b2_nonzero):
    import concourse.bacc as bacc
    import concourse.mybir as mybir
    from concourse.tile import TileContext

    f32 = mybir.dt.float32
    nc = bacc.Bacc("TRN2", target_bir_lowering=False, debug=False,
                   num_devices=NCORES)
    XT = nc.declare_dram_parameter("XT", [DPC * NP1, B], f32, isOutput=False)
    WD = nc.declare_dram_parameter("WD", [N, NP1, 4 * H], f32, isOutput=False)
    W2D = nc.declare_dram_parameter("W2D", [4 * H, 4 * N], f32, isOutput=False)
    B2D = nc.declare_dram_parameter("B2D", [DPC, N], f32, isOutput=False)
    OUT = nc.declare_dram_parameter("OUT", [N, DPC * B], f32, isOutput=True)

    XTr = XT.rearrange("(d p) b -> d p b", d=DPC)

    with TileContext(nc) as tc:
        with (
            tc.tile_pool(name="state", bufs=1) as state_pool,
            tc.tile_pool(name="aux", bufs=1) as aux_pool,
            tc.tile_pool(name="wts", bufs=4) as w_pool,
            tc.tile_pool(name="hsb", bufs=3) as h_pool,
            tc.tile_pool(name="outsb", bufs=3) as o_pool,
            tc.tile_pool(name="ph", bufs=3, space="PSUM") as ph_pool,
            tc.tile_pool(name="po", bufs=2, space="PSUM") as po_pool,
        ):
            Sall = state_pool.tile([NP1, DPC * B], f32, name="Sall")
            for d in range(DPC):
                nc.sync.dma_start(
                    out=Sall[:, d * B : (d + 1) * B], in_=XTr[d, :, :]
                )
            W2sb = aux_pool.tile([4 * H, 4 * N], f32, name="W2sb")
            nc.sync.dma_start(out=W2sb[:, :], in_=W2D[:, :])
            b2sb = aux_pool.tile([DPC, N], f32, name="b2sb")
            nc.sync.dma_start(out=b2sb[:, :], in_=B2D[:, :])

            for t in range(N):
                wt = w_pool.tile([NP1, 4 * H], f32, tag="wt", name="wt")
                nc.sync.dma_start(out=wt[:, :], in_=WD[t, :, :])
                for c in range(CH):
                    if c % 2 == 0:
                        op = po_pool.tile([DPC, 2 * CW], f32, tag="op", name="op")
                    hp = ph_pool.tile([4 * H, CW], f32, tag="hp", name="hp")
                    for d in range(DPC):
                        nc.tensor.matmul(
                            hp[32 * d : 32 * (d + 1), :],
                            lhsT=wt[:, 32 * d : 32 * (d + 1)],
                            rhs=Sall[:, d * B + c * CW : d * B + (c + 1) * CW],
                            start=True,
                            stop=True,
                            tile_position=(0, 32 * d),
                        )
                    hs = h_pool.tile([4 * H, CW], f32, tag="hs", name="hs")
                    nc.scalar.activation(
                        hs[:, :], hp[:, :],
                        mybir.ActivationFunctionType.Lrelu,
                        bias=0.0, scale=1.0, alpha=0.01,
                    )
                    nc.tensor.matmul(
                        op[:, (c % 2) * CW : (c % 2 + 1) * CW],
                        lhsT=W2sb[:, 4 * t : 4 * t + 4],
                        rhs=hs[:, :],
                        start=True,
                        stop=True,
                    )
                    if c % 2 == 1:
                        half = c // 2
                        osb = o_pool.tile([DPC, 2 * CW], f32, tag="osb", name="osb")
                        if b2_nonzero:
                            nc.vector.tensor_scalar_add(
                                osb[:, :], op[:, :], b2sb[:, t : t + 1]
                            )
                        else:
                            nc.vector.tensor_copy(osb[:, :], op[:, :])
                        # scatter the 4 dag rows into Sall row t in one DMA:
                        # dest free-dim AP = [dag-stride B, count 4][1, 2*CW]
                        dst = Sall[t : t + 1, :].rearrange(
                            "r (d b) -> r d b", d=DPC
                        )[:, :, half * 2 * CW : (half + 1) * 2 * CW]
                        nc.sync.dma_start(out=dst, in_=osb[:, :])
            nc.sync.dma_start(out=OUT[:, :], in_=Sall[:N, :])
    nc.compile()
    return nc


def kernel(x, A, order, do_idx, W1, b1, W2, b2):
    x = np.asarray(x, np.float32)
    A = np.asarray(A, np.float32)
    order = np.asarray(order)
    W1 = np.asarray(W1, np.float32)
    b1 = np.asarray(b1, np.float32)
    W2 = np.asarray(W2, np.float32)
    b2 = np.asarray(b2, np.float32)
    do = int(np.asarray(do_idx))

    packs = [_pack_core(k, x, A, order, do, W1, b1, W2, b2)
             for k in range(NCORES)]

    if os.environ.get("KERNEL_NUMPY_EMU"):
        outs = [_emulate_core(m) for m in packs]
    else:
        key = bool(np.any(b2 != 0.0))
        if key not in _PROGRAM_CACHE:
            _PROGRAM_CACHE[key] = _build_program(key)
        nc = _PROGRAM_CACHE[key]
        from concourse.bass_utils import run_bass_kernel_spmd

        in_maps = [
            {"XT": m["XT"], "WD": m["WD"], "W2D": m["W2D"], "B2D": m["B2D"]}
            for m in packs
        ]
        res = run_bass_kernel_spmd(nc, in_maps, list(range(NCORES)))
        outs = [res.results[k]["OUT"] for k in range(NCORES)]

    out_full = np.empty((D, B, N), np.float32)
    for k in range(NCORES):
        O = outs[k]  # [N(topo rows), DPC*B]
        for d in range(DPC):
            dag = k * DPC + d
            od = order[dag].astype(np.int64)
            out_full[dag][:, od] = O[:, d * B : (d + 1) * B].T
    return out_full


# revision 4
# speedup vs baseline: 1.1233x; 1.1233x over previous
"""Trainium2 Bass kernel for nn_Conditionals (DAG-MLP scan).

Strategy
--------
Shard the D=32 dags across 8 cores (4 dags/core). Per dag, keep the
scan state transposed in SBUF as S[p, b] (node-slot on partitions,
batch on the free dim), stored *in topological-order coordinates*:
physical row p holds the value of node order[d][p]. In these
coordinates every step t writes row t, so the instruction stream is
identical across cores (SPMD) — all data dependence is folded into
host-precomputed per-step weights.

Per step t (node n = order[d][t]) the reference computes
    h   = leaky_relu(concat(outputs * mask, x[:, n]) @ W1[n].T + b1[n])
    out = h @ W2[n] + b2[n]
With the state seeded as S[p] = x[:, order[p]] (value of an
uncomputed node IS its x column, consumed via W1's x-column weight)
this collapses to one K=101 matmul per step:
    Wf[p, h] = A[od[p], od[t]] * (p < t) * W1[n][h, od[p]]   (p < 100)
    Wf[t, h] = W1[n][h, 100]          (x column; row t still holds x)
    Wf[100, h] = b1[n]                (constant-ones state row 100)
    h = leaky_relu(Wf.T @ S)
The do_idx intervention step writes back x[:, do_idx], reproduced
exactly from the seed row through the two-channel identity
    x = c0*lrelu(x) + c1*lrelu(-x),  c0 = -c1 = 0.99/0.9999,
keeping the program uniform across dags/cores.

Device layout per core: one state tensor Sall[101, 4*8192] (4 dags
side by side), per-step folded weights streamed from DRAM, layer-1
into PSUM [128, 512] (4 dags x H=32 stacked via PE column tiling),
LeakyReLU on the scalar engine, layer-2 with a block-diagonal W2
[128, 4], PSUM->SBUF copy (+b2) on the vector engine, and one DMA per
half-step scattering the 4 dag rows back into Sall row t.
"""

import os
import sys

sys.path.insert(0, "/opt/trn_rl_repo")

import numpy as np

B = 8192
N = 100
D = 32
H = 32
NCORES = 8
DPC = D // NCORES  # dags per core
CW = 512           # matmul chunk width (one PSUM bank of fp32)
CH = B // CW       # chunks per dag
NP1 = N + 1        # state rows incl. constant-ones row

C0 = np.float32(0.99 / 0.9999)  # lrelu(x), lrelu(-x) -> x reconstruction


def _pack_core(core, x, A, order, do_idx, W1, b1, W2, b2):
    """Host-side fold of masks/params into per-step arrays for one core."""
    xT = np.ascontiguousarray(x.T.astype(np.float32))  # [N, B]
    WD = np.zeros((N, NP1, 4 * H), np.float32)    # [t, p, 32d+h]
    W2D = np.zeros((4 * H, 4 * N), np.float32)    # [32d+h, 4t+d]
    B2D = np.zeros((DPC, N), np.float32)          # [d, t]
    XT = np.empty((DPC, NP1, B), np.float32)      # [d, p(+ones), b]
    tri = np.tril(np.ones((N, N), np.float32), k=-1)  # [t, p] strict lower
    idx = np.arange(N)
    for d in range(DPC):
        dag = core * DPC + d
        od = order[dag].astype(np.int64)
        Aperm = A[dag][np.ix_(od, od)]            # [p, t]
        W1p = W1[od]                              # [t, H, N+1]
        W1sel = W1p[:, :, :N][:, :, od]           # [t, h, p]
        M = Aperm.T * tri                         # [t, p]
        Wf = np.einsum("tp,thp->tph", M, W1sel)   # [t, p, h]
        Wf[idx, idx, :] = W1p[:, :, N]            # x-column weight on diag
        full = np.zeros((N, NP1, H), np.float32)
        full[:, :N, :] = Wf
        full[:, N, :] = b1[od]                    # ones-row -> +b1
        W2v = W2[od].astype(np.float32).copy()    # [t, H]
        b2v = b2[od].astype(np.float32).copy()    # [t]
        t0 = int(np.where(od == do_idx)[0][0])    # the intervention step
        full[t0] = 0.0
        full[t0, t0, 0] = 1.0
        full[t0, t0, 1] = -1.0
        W2v[t0] = 0.0
        W2v[t0, 0] = C0
        W2v[t0, 1] = -C0
        b2v[t0] = 0.0
        WD[:, :, 32 * d : 32 * (d + 1)] = full
        W2D[32 * d : 32 * (d + 1), 4 * idx + d] = W2v.T
        B2D[d] = b2v
        XT[d, :N, :] = xT[od]
        XT[d, N, :] = 1.0
    return {
        "XT": np.ascontiguousarray(XT.reshape(DPC * NP1, B)),
        "WD": WD,
        "W2D": W2D,
        "B2D": B2D,
    }


def _emulate_core(m):
    """Numpy emulation of the device program (packing validation)."""
    S = m["XT"].reshape(DPC, NP1, B).transpose(1, 0, 2).reshape(NP1, DPC * B).copy()
    # device Sall layout: [101, 4*B] with dag-major free dim
    S = np.empty((NP1, DPC * B), np.float32)
    XT = m["XT"].reshape(DPC, NP1, B)
    for d in range(DPC):
        S[:, d * B : (d + 1) * B] = XT[d]
    for t in range(N):
        wt = m["WD"][t]                           # [101, 128]
        for d in range(DPC):
            rhs = S[:, d * B : (d + 1) * B]       # [101, B]
            pre = wt[:, 32 * d : 32 * (d + 1)].T @ rhs  # [32, B]
            h = np.where(pre >= 0, pre, np.float32(0.01) * pre)
            w2col = m["W2D"][:, 4 * t + d][32 * d : 32 * (d + 1)]
            out2 = w2col @ h + m["B2D"][d, t]
            S[t, d * B : (d + 1) * B] = out2
    return S[:N]  # [100, 4*B]


_PROGRAM_CACHE = {}


def _build_program(b2_nonzero):
    import concourse.bacc as bacc
    import concourse.mybir as mybir
    from concourse.tile import TileContext

    f32 = mybir.dt.float32
    nc = bacc.Bacc("TRN2", target_bir_lowering=False, debug=False,
                   num_devices=NCORES)
    XT = nc.declare_dram_parameter("XT", [DPC * NP1, B], f32, isOutput=False)
    WD = nc.declare_dram_parameter("WD", [N, NP1, 4 * H], f32, isOutput=False)
    W2D = nc.declare_dram_parameter("W2D", [4 * H, 4 * N], f32, isOutput=False)
    B2D = nc.declare_dram_parameter("B2D", [DPC, N], f32, isOutput=False)
    OUT = nc.declare_dram_parameter("OUT", [N, DPC * B], f32, isOutput=True)

    XTr = XT.rearrange("(d p) b -> d p b", d=DPC)

    with TileContext(nc) as tc:
        with (
            tc.tile_pool(name="state", bufs=1) as state_pool,
            tc.tile_pool(name="aux", bufs=1) as aux_pool,
            tc.tile_pool(name="wts", bufs=4) as w_pool,
            tc.tile_pool(name="hsb", bufs=4) as h_pool,
            tc.tile_pool(name="outsb", bufs=3) as o_pool,
            tc.tile_pool(name="ph", bufs=3, space="PSUM") as ph_pool,
            tc.tile_pool(name="po", bufs=2, space="PSUM") as po_pool,
        ):
            Sall = state_pool.tile([NP1, DPC * B], f32, name="Sall")
            for d in range(DPC):
                nc.sync.dma_start(
                    out=Sall[:, d * B : (d + 1) * B], in_=XTr[d, :, :]
                )
            W2sb = aux_pool.tile([4 * H, 4 * N], f32, name="W2sb")
            nc.sync.dma_start(out=W2sb[:, :], in_=W2D[:, :])
            b2sb = aux_pool.tile([DPC, N], f32, name="b2sb")
            nc.sync.dma_start(out=b2sb[:, :], in_=B2D[:, :])

            # Software-pipelined chunk stream: the layer-2 matmul for chunk
            # g runs LAG L1-groups later, so the in-order PE never stalls
            # on the LeakyReLU that produces its rhs.
            LAG = 2
            pend = []  # (t, c, hs_tile, op_tile) awaiting layer-2 emission

            def emit_l2(tc_, t, c, hs, op):
                nc.tensor.matmul(
                    op[:, (c % 2) * CW : (c % 2 + 1) * CW],
                    lhsT=W2sb[:, 4 * t : 4 * t + 4],
                    rhs=hs[:, :],
                    start=True,
                    stop=True,
                )
                if c % 2 == 1:
                    half = c // 2
                    osb = o_pool.tile([DPC, 2 * CW], f32, tag="osb", name="osb")
                    if b2_nonzero:
                        nc.vector.tensor_scalar_add(
                            osb[:, :], op[:, :], b2sb[:, t : t + 1]
                        )
                    else:
                        nc.vector.tensor_copy(osb[:, :], op[:, :])
                    # scatter the 4 dag rows into Sall row t in one DMA:
                    # dest free-dim AP = [dag-stride B, count 4][1, 2*CW]
                    dst = Sall[t : t + 1, :].rearrange(
                        "r (d b) -> r d b", d=DPC
                    )[:, :, half * 2 * CW : (half + 1) * 2 * CW]
                    nc.sync.dma_start(out=dst, in_=osb[:, :])

            op = None
            for t in range(N):
                wt = w_pool.tile([NP1, 4 * H], f32, tag="wt", name="wt")
                nc.sync.dma_start(out=wt[:, :], in_=WD[t, :, :])
                for c in range(CH):
                    if c % 2 == 0:
                        op = po_pool.tile([DPC, 2 * CW], f32, tag="op", name="op")
                    hp = ph_pool.tile([4 * H, CW], f32, tag="hp", name="hp")
                    for d in range(DPC):
                        nc.tensor.matmul(
                            hp[32 * d : 32 * (d + 1), :],
                            lhsT=wt[:, 32 * d : 32 * (d + 1)],
                            rhs=Sall[:, d * B + c * CW : d * B + (c + 1) * CW],
                            start=True,
                            stop=True,
                            tile_position=(0, 32 * d),
                        )
                    hs = h_pool.tile([4 * H, CW], f32, tag="hs", name="hs")
                    nc.scalar.activation(
                        hs[:, :], hp[:, :],
                        mybir.ActivationFunctionType.Lrelu,
                        bias=0.0, scale=1.0, alpha=0.01,
                    )
                    pend.append((t, c, hs, op))
                    if len(pend) > LAG:
                        emit_l2(tc, *pend.pop(0))
            while pend:
                emit_l2(tc, *pend.pop(0))
            nc.sync.dma_start(out=OUT[:, :], in_=Sall[:N, :])
    nc.compile()
    return nc


def kernel(x, A, order, do_idx, W1, b1, W2, b2):
    x = np.asarray(x, np.float32)
    A = np.asarray(A, np.float32)
    order = np.asarray(order)
    W1 = np.asarray(W1, np.float32)
    b1 = np.asarray(b1, np.float32)
    W2 = np.asarray(W2, np.float32)
    b2 = np.asarray(b2, np.float32)
    do = int(np.asarray(do_idx))

    packs = [_pack_core(k, x, A, order, do, W1, b1, W2, b2)
             for k in range(NCORES)]

    if os.environ.get("KERNEL_NUMPY_EMU"):
        outs = [_emulate_core(m) for m in packs]
    else:
        key = bool(np.any(b2 != 0.0))
        if key not in _PROGRAM_CACHE:
            _PROGRAM_CACHE[key] = _build_program(key)
        nc = _PROGRAM_CACHE[key]
        from concourse.bass_utils import run_bass_kernel_spmd

        in_maps = [
            {"XT": m["XT"], "WD": m["WD"], "W2D": m["W2D"], "B2D": m["B2D"]}
            for m in packs
        ]
        res = run_bass_kernel_spmd(nc, in_maps, list(range(NCORES)))
        outs = [res.results[k]["OUT"] for k in range(NCORES)]

    out_full = np.empty((D, B, N), np.float32)
    for k in range(NCORES):
        O = outs[k]  # [N(topo rows), DPC*B]
        for d in range(DPC):
            dag = k * DPC + d
            od = order[dag].astype(np.int64)
            out_full[dag][:, od] = O[:, d * B : (d + 1) * B].T
    return out_full


# revision 9
# speedup vs baseline: 1.8269x; 1.6264x over previous
"""Trainium2 Bass kernel for nn_Conditionals (DAG-MLP scan).

Strategy
--------
Shard the D=32 dags across 8 cores (4 dags/core). Per dag, keep the
scan state transposed in SBUF as S[p, b] (node-slot on partitions,
batch on the free dim), stored *in topological-order coordinates*:
physical row p holds the value of node order[d][p]. In these
coordinates every step t writes row t, so the instruction stream is
identical across cores (SPMD) — all data dependence is folded into
host-precomputed per-step weights.

Per step t (node n = order[d][t]) the reference computes
    h   = leaky_relu(concat(outputs * mask, x[:, n]) @ W1[n].T + b1[n])
    out = h @ W2[n] + b2[n]
With the state seeded as S[p] = x[:, order[p]] (value of an
uncomputed node IS its x column, consumed via W1's x-column weight)
this collapses to one K=101 matmul per step:
    Wf[p, h] = A[od[p], od[t]] * (p < t) * W1[n][h, od[p]]   (p < 100)
    Wf[t, h] = W1[n][h, 100]          (x column; row t still holds x)
    Wf[100, h] = b1[n]                (constant-ones state row 100)
    h = leaky_relu(Wf.T @ S)
The do_idx intervention step writes back x[:, do_idx], reproduced
exactly from the seed row through the two-channel identity
    x = c0*lrelu(x) + c1*lrelu(-x),  c0 = -c1 = 0.99/0.9999,
keeping the program uniform across dags/cores.

Device layout per core: one state tensor Sall[101, 4*8192] (4 dags
side by side), per-step folded weights streamed from DRAM, layer-1
into PSUM [128, 512] (4 dags x H=32 stacked via PE column tiling),
LeakyReLU on the scalar engine, layer-2 with a block-diagonal W2
[128, 4], PSUM->SBUF copy (+b2) on the vector engine, and one DMA per
half-step scattering the 4 dag rows back into Sall row t.
"""

import os
import sys

sys.path.insert(0, "/opt/trn_rl_repo")

import numpy as np

B = 8192
N = 100
D = 32
H = 32
NCORES = 8
DPC = D // NCORES  # dags per core
CW = 512           # matmul chunk width (one PSUM bank of fp32)
CH = B // CW       # chunks per dag
NP1 = N + 1        # state rows incl. constant-ones row

C0 = np.float32(0.99 / 0.9999)  # lrelu(x), lrelu(-x) -> x reconstruction


def _pack_core(core, x, A, order, do_idx, W1, b1, W2, b2):
    """Host-side fold of masks/params into per-step arrays for one core."""
    xT = np.ascontiguousarray(x.T.astype(np.float32))  # [N, B]
    WD = np.zeros((N, NP1, 4 * H), np.float32)    # [t, p, 32d+h]
    W2D = np.zeros((4 * H, 4 * N), np.float32)    # [32d+h, 4t+d]
    B2D = np.zeros((DPC, N), np.float32)          # [d, t]
    XT = np.empty((DPC, NP1, B), np.float32)      # [d, p(+ones), b]
    tri = np.tril(np.ones((N, N), np.float32), k=-1)  # [t, p] strict lower
    idx = np.arange(N)
    for d in range(DPC):
        dag = core * DPC + d
        od = order[dag].astype(np.int64)
        Aperm = A[dag][np.ix_(od, od)]            # [p, t]
        W1p = W1[od]                              # [t, H, N+1]
        W1sel = W1p[:, :, :N][:, :, od]           # [t, h, p]
        M = Aperm.T * tri                         # [t, p]
        Wf = np.einsum("tp,thp->tph", M, W1sel)   # [t, p, h]
        Wf[idx, idx, :] = W1p[:, :, N]            # x-column weight on diag
        full = np.zeros((N, NP1, H), np.float32)
        full[:, :N, :] = Wf
        full[:, N, :] = b1[od]                    # ones-row -> +b1
        W2v = W2[od].astype(np.float32).copy()    # [t, H]
        b2v = b2[od].astype(np.float32).copy()    # [t]
        t0 = int(np.where(od == do_idx)[0][0])    # the intervention step
        full[t0] = 0.0
        full[t0, t0, 0] = 1.0
        full[t0, t0, 1] = -1.0
        W2v[t0] = 0.0
        W2v[t0, 0] = C0
        W2v[t0, 1] = -C0
        b2v[t0] = 0.0
        WD[:, :, 32 * d : 32 * (d + 1)] = full
        W2D[32 * d : 32 * (d + 1), 4 * idx + d] = W2v.T
        B2D[d] = b2v
        XT[d, :N, :] = xT[od]
        XT[d, N, :] = 1.0
    # b2 replicated for the 4-chunk-stacked out2 tile: row 32j+d -> b2[d]
    B2R = np.zeros((128, N), np.float32)
    for j in range(4):
        B2R[32 * j : 32 * j + DPC, :] = B2D
    import ml_dtypes
    return {
        "XT": np.ascontiguousarray(XT.reshape(DPC * NP1, B)),
        "WD": WD,
        "W2D": W2D.astype(ml_dtypes.bfloat16),
        "B2D": B2R,
    }


def _emulate_core(m):
    """Numpy emulation of the device program (packing validation)."""
    S = m["XT"].reshape(DPC, NP1, B).transpose(1, 0, 2).reshape(NP1, DPC * B).copy()
    # device Sall layout: [101, 4*B] with dag-major free dim
    S = np.empty((NP1, DPC * B), np.float32)
    XT = m["XT"].reshape(DPC, NP1, B)
    for d in range(DPC):
        S[:, d * B : (d + 1) * B] = XT[d]
    for t in range(N):
        wt = m["WD"][t]                           # [101, 128]
        for d in range(DPC):
            rhs = S[:, d * B : (d + 1) * B]       # [101, B]
            import ml_dtypes
            pre = wt[:, 32 * d : 32 * (d + 1)].T @ rhs  # [32, B]
            h = np.where(pre >= 0, pre, np.float32(0.01) * pre)
            h = h.astype(ml_dtypes.bfloat16).astype(np.float32)
            w2col = m["W2D"][:, 4 * t + d][32 * d : 32 * (d + 1)].astype(np.float32)
            out2 = w2col @ h + m["B2D"][d, t]
            S[t, d * B : (d + 1) * B] = out2
    return S[:N]  # [100, 4*B]


_PROGRAM_CACHE = {}


def _build_program(b2_nonzero):
    import concourse.bacc as bacc
    import concourse.mybir as mybir
    from concourse.tile import TileContext

    f32 = mybir.dt.float32
    bf16 = mybir.dt.bfloat16
    nc = bacc.Bacc("TRN2", target_bir_lowering=False, debug=False,
                   num_devices=NCORES)
    XT = nc.declare_dram_parameter("XT", [DPC * NP1, B], f32, isOutput=False)
    WD = nc.declare_dram_parameter("WD", [N, NP1, 4 * H], f32, isOutput=False)
    W2D = nc.declare_dram_parameter("W2D", [4 * H, 4 * N], bf16, isOutput=False)
    B2D = nc.declare_dram_parameter("B2D", [4 * H, N], f32, isOutput=False)
    OUT = nc.declare_dram_parameter("OUT", [N, DPC * B], f32, isOutput=True)

    XTr = XT.rearrange("(d p) b -> d p b", d=DPC)

    with TileContext(nc) as tc:
        with (
            tc.tile_pool(name="state", bufs=1) as state_pool,
            tc.tile_pool(name="aux", bufs=1) as aux_pool,
            tc.tile_pool(name="wts", bufs=4) as w_pool,
            tc.tile_pool(name="hsb", bufs=4) as h_pool,
            tc.tile_pool(name="outsb", bufs=3) as o_pool,
            tc.tile_pool(name="ph", bufs=3, space="PSUM") as ph_pool,
            tc.tile_pool(name="po", bufs=2, space="PSUM") as po_pool,
        ):
            Sall = state_pool.tile([NP1, DPC * B], f32, name="Sall")
            for d in range(DPC):
                nc.sync.dma_start(
                    out=Sall[:, d * B : (d + 1) * B], in_=XTr[d, :, :]
                )
            W2sb = aux_pool.tile([4 * H, 4 * N], bf16, name="W2sb")
            nc.sync.dma_start(out=W2sb[:, :], in_=W2D[:, :])
            b2sb = aux_pool.tile([4 * H, N], f32, name="b2sb")
            nc.sync.dma_start(out=b2sb[:, :], in_=B2D[:, :])

            # Software-pipelined chunk stream: the layer-2 matmul for chunk
            # g runs LAG L1-groups later, so the in-order PE never stalls
            # on the LeakyReLU that produces its rhs.
            LAG = 2
            pend = []  # (t, c, hs_tile, op_tile) awaiting layer-2 emission

            def emit_l2(tc_, t, c, hs, op):
                # stack chunk c's [4, CW] out2 at partition 32*(c%4) so one
                # DVE copy + 4 strided DMAs retire 4 chunks at once
                j = c % 4
                nc.tensor.matmul(
                    op[32 * j : 32 * j + DPC, :],
                    lhsT=W2sb[:, 4 * t : 4 * t + 4],
                    rhs=hs[:, :],
                    start=True,
                    stop=True,
                    tile_position=(0, 32 * j),
                )
                if j == 3:
                    q = c // 4
                    osb = o_pool.tile([4 * H, CW], f32, tag="osb", name="osb")
                    if b2_nonzero:
                        nc.vector.tensor_scalar_add(
                            osb[:, :], op[:, :], b2sb[:, t : t + 1]
                        )
                    else:
                        nc.vector.tensor_copy(osb[:, :], op[:, :])
                    osr = osb.rearrange("(j d) w -> j d w", j=4)
                    for d in range(DPC):
                        nc.sync.dma_start(
                            out=Sall[t : t + 1,
                                     d * B + q * 4 * CW : d * B + (q + 1) * 4 * CW],
                            in_=osr[:, d, :],
                        )

            op = None
            for t in range(N):
                wt = w_pool.tile([NP1, 4 * H], f32, tag="wt", name="wt")
                nc.sync.dma_start(out=wt[:, :], in_=WD[t, :, :])
                for c in range(CH):
                    if c % 4 == 0:
                        op = po_pool.tile([4 * H, CW], f32, tag="op", name="op")
                    hp = ph_pool.tile([4 * H, CW], f32, tag="hp", name="hp")
                    for d in range(DPC):
                        nc.tensor.matmul(
                            hp[32 * d : 32 * (d + 1), :],
                            lhsT=wt[:, 32 * d : 32 * (d + 1)],
                            rhs=Sall[:, d * B + c * CW : d * B + (c + 1) * CW],
                            start=True,
                            stop=True,
                            tile_position=(0, 32 * d),
                        )
                    hs = h_pool.tile([4 * H, CW], bf16, tag="hs", name="hs")
                    nc.scalar.activation(
                        hs[:, :], hp[:, :],
                        mybir.ActivationFunctionType.Lrelu,
                        bias=0.0, scale=1.0, alpha=0.01,
                    )
                    pend.append((t, c, hs, op))
                    if len(pend) > LAG:
                        emit_l2(tc, *pend.pop(0))
            while pend:
                emit_l2(tc, *pend.pop(0))
            nc.sync.dma_start(out=OUT[:, :], in_=Sall[:N, :])
    nc.compile()
    return nc


def kernel(x, A, order, do_idx, W1, b1, W2, b2):
    x = np.asarray(x, np.float32)
    A = np.asarray(A, np.float32)
    order = np.asarray(order)
    W1 = np.asarray(W1, np.float32)
    b1 = np.asarray(b1, np.float32)
    W2 = np.asarray(W2, np.float32)
    b2 = np.asarray(b2, np.float32)
    do = int(np.asarray(do_idx))

    packs = [_pack_core(k, x, A, order, do, W1, b1, W2, b2)
             for k in range(NCORES)]

    if os.environ.get("KERNEL_NUMPY_EMU"):
        outs = [_emulate_core(m) for m in packs]
    else:
        key = bool(np.any(b2 != 0.0))
        if key not in _PROGRAM_CACHE:
            _PROGRAM_CACHE[key] = _build_program(key)
        nc = _PROGRAM_CACHE[key]
        from concourse.bass_utils import run_bass_kernel_spmd

        in_maps = [
            {"XT": m["XT"], "WD": m["WD"], "W2D": m["W2D"], "B2D": m["B2D"]}
            for m in packs
        ]
        res = run_bass_kernel_spmd(nc, in_maps, list(range(NCORES)))
        outs = [res.results[k]["OUT"] for k in range(NCORES)]

    out_full = np.empty((D, B, N), np.float32)
    for k in range(NCORES):
        O = outs[k]  # [N(topo rows), DPC*B]
        for d in range(DPC):
            dag = k * DPC + d
            od = order[dag].astype(np.int64)
            out_full[dag][:, od] = O[:, d * B : (d + 1) * B].T
    return out_full


# revision 10
# speedup vs baseline: 1.9421x; 1.0631x over previous
"""Trainium2 Bass kernel for nn_Conditionals (DAG-MLP scan).

Strategy
--------
Shard the D=32 dags across 8 cores (4 dags/core). Per dag, keep the
scan state transposed in SBUF as S[p, b] (node-slot on partitions,
batch on the free dim), stored *in topological-order coordinates*:
physical row p holds the value of node order[d][p]. In these
coordinates every step t writes row t, so the instruction stream is
identical across cores (SPMD) — all data dependence is folded into
host-precomputed per-step weights.

Per step t (node n = order[d][t]) the reference computes
    h   = leaky_relu(concat(outputs * mask, x[:, n]) @ W1[n].T + b1[n])
    out = h @ W2[n] + b2[n]
With the state seeded as S[p] = x[:, order[p]] (the value of a
not-yet-computed node IS its x column, consumed via W1's x-column
weight) this collapses to one K=101 matmul per step:
    Wf[p, h] = A[od[p], od[t]] * (p < t) * W1[n][h, od[p]]   (p < 100)
    Wf[t, h] = W1[n][h, 100]          (x column; row t still holds x)
    Wf[100, h] = b1[n]                (constant-ones state row 100)
    h = leaky_relu(Wf.T @ S)
The do_idx intervention step must write back x[:, do_idx]; it is
reproduced from the seed row through the identity
    x = w*lrelu(g*x) - w*lrelu(-g*x),  g*w = 1/(1+alpha),
with (g, w) chosen on the bf16 grid so their fp32 product is exact to
~2^-16. This keeps the program uniform across dags/cores.

Device schedule per core: one state tensor Sall[101, 4*8192] (4 dags
side by side), per-step folded weights streamed from DRAM; layer-1
PSUM tiles [128, 1024] hold two 512-chunks for 4 dags (PE column
tiling); LeakyReLU runs on the scalar engine for most chunk-pairs and
as mul+max on the vector engine for the rest (load balance); layer-2
uses a block-diagonal W2 [128, 4] with chunk out2 stacked 4-high in
one PSUM bank at 32-aligned offsets, so a single [128, 512] DVE copy
plus 4 partition-strided DMAs retire four chunks of state writeback
at once. Layer-2 matmuls trail layer-1 by two chunk-pairs so the
in-order PE never waits on an activation.
"""

import os
import sys

sys.path.insert(0, "/opt/trn_rl_repo")

import ml_dtypes
import numpy as np

B = 8192
N = 100
D = 32
H = 32
NCORES = 8
DPC = D // NCORES  # dags per core
CW = 512           # matmul chunk width (one PSUM bank of fp32)
CH = B // CW       # chunks per dag per step
NP1 = N + 1        # state rows incl. constant-ones row
ALPHA = np.float32(0.01)

BF16 = ml_dtypes.bfloat16


def _do_step_coeffs(np_dtype):
    """(g, w) with w*lrelu(g*x) - w*lrelu(-g*x) == x, exact in fp32 accum."""
    target = np.float64(1.0) / (1.0 + np.float64(ALPHA))
    if np_dtype == np.float32:
        return np.float32(1.0), np.float32(target)
    best = None
    for i in range(256):
        g = np.float32(np_dtype(np.float32(1.0 + i / 256.0)))
        w = np.float32(np_dtype(np.float32(target) / g))
        err = abs(np.float64(g) * np.float64(w) - target)
        if best is None or err < best[0]:
            best = (err, g, w)
    return best[1], best[2]


def _pack_core(core, x, A, order, do_idx, W1, b1, W2, b2, np_dtype):
    """Host-side fold of masks/params into per-step arrays for one core."""
    xT = np.ascontiguousarray(x.T.astype(np.float32))  # [N, B]
    g, w = _do_step_coeffs(np_dtype)
    WD = np.zeros((N, NP1, 4 * H), np.float32)    # [t, p, 32d+h]
    W2D = np.zeros((4 * H, 4 * N), np.float32)    # [32d+h, 4t+d]
    B2D = np.zeros((DPC, N), np.float32)          # [d, t]
    XT = np.empty((DPC, NP1, B), np.float32)      # [d, p(+ones), b]
    tri = np.tril(np.ones((N, N), np.float32), k=-1)  # [t, p] strict lower
    idx = np.arange(N)
    for d in range(DPC):
        dag = core * DPC + d
        od = order[dag].astype(np.int64)
        Aperm = A[dag][np.ix_(od, od)]            # [p, t]
        W1p = W1[od]                              # [t, H, N+1]
        W1sel = W1p[:, :, :N][:, :, od]           # [t, h, p]
        M = Aperm.T * tri                         # [t, p]
        Wf = np.einsum("tp,thp->tph", M, W1sel)   # [t, p, h]
        Wf[idx, idx, :] = W1p[:, :, N]            # x-column weight on diag
        full = np.zeros((N, NP1, H), np.float32)
        full[:, :N, :] = Wf
        full[:, N, :] = b1[od]                    # ones-row -> +b1
        W2v = W2[od].astype(np.float32).copy()    # [t, H]
        b2v = b2[od].astype(np.float32).copy()    # [t]
        t0 = int(np.where(od == do_idx)[0][0])    # the intervention step
        full[t0] = 0.0
        full[t0, t0, 0] = g
        full[t0, t0, 1] = -g
        W2v[t0] = 0.0
        W2v[t0, 0] = w
        W2v[t0, 1] = -w
        b2v[t0] = 0.0
        WD[:, :, 32 * d : 32 * (d + 1)] = full
        W2D[32 * d : 32 * (d + 1), 4 * idx + d] = W2v.T
        B2D[d] = b2v
        XT[d, :N, :] = xT[od]
        XT[d, N, :] = 1.0
    # b2 replicated for the 4-chunk-stacked out2 tile: row 32j+d -> b2[d]
    B2R = np.zeros((128, N), np.float32)
    for j in range(4):
        B2R[32 * j : 32 * j + DPC, :] = B2D
    return {
        "XT": np.ascontiguousarray(XT.reshape(DPC * NP1, B)).astype(np_dtype),
        "WD": WD.astype(np_dtype),
        "W2D": W2D.astype(np_dtype),
        "B2D": B2R,
    }


def _emulate_core(m):
    """Numpy emulation of the device program (packing validation)."""
    dt = m["XT"].dtype
    XT = m["XT"].reshape(DPC, NP1, B)
    S = np.empty((NP1, DPC * B), dt)
    for d in range(DPC):
        S[:, d * B : (d + 1) * B] = XT[d]
    for t in range(N):
        wt = m["WD"][t].astype(np.float32)        # [101, 128]
        for d in range(DPC):
            rhs = S[:, d * B : (d + 1) * B].astype(np.float32)
            pre = wt[:, 32 * d : 32 * (d + 1)].T @ rhs  # [32, B] fp32 accum
            h = np.where(pre >= 0, pre, ALPHA * pre).astype(dt)
            w2col = m["W2D"][:, 4 * t + d][32 * d : 32 * (d + 1)]
            out2 = w2col.astype(np.float32) @ h.astype(np.float32)
            out2 += m["B2D"][d, t]
            S[t, d * B : (d + 1) * B] = out2.astype(dt)
    return S[:N].astype(np.float32)  # [100, 4*B]


_PROGRAM_CACHE = {}


def _build_program(b2_nonzero, np_dtype):
    import concourse.bacc as bacc
    import concourse.mybir as mybir
    from concourse.tile import TileContext

    f32 = mybir.dt.float32
    DT = mybir.dt.bfloat16 if np_dtype == BF16 else f32
    nc = bacc.Bacc("TRN2", target_bir_lowering=False, debug=False,
                   num_devices=NCORES)
    XT = nc.declare_dram_parameter("XT", [DPC * NP1, B], DT, isOutput=False)
    WD = nc.declare_dram_parameter("WD", [N, NP1, 4 * H], DT, isOutput=False)
    W2D = nc.declare_dram_parameter("W2D", [4 * H, 4 * N], DT, isOutput=False)
    B2D = nc.declare_dram_parameter("B2D", [4 * H, N], f32, isOutput=False)
    OUT = nc.declare_dram_parameter("OUT", [N, DPC * B], DT, isOutput=True)

    XTr = XT.rearrange("(d p) b -> d p b", d=DPC)

    with TileContext(nc) as tc:
        with (
            tc.tile_pool(name="state", bufs=1) as state_pool,
            tc.tile_pool(name="aux", bufs=1) as aux_pool,
            tc.tile_pool(name="wts", bufs=6) as w_pool,
            tc.tile_pool(name="hsb", bufs=3) as h_pool,
            tc.tile_pool(name="tmp", bufs=2) as tmp_pool,
            tc.tile_pool(name="outsb", bufs=3) as o_pool,
            tc.tile_pool(name="ph", bufs=3, space="PSUM") as ph_pool,
            tc.tile_pool(name="po", bufs=2, space="PSUM") as po_pool,
        ):
            Sall = state_pool.tile([NP1, DPC * B], DT, name="Sall")
            for d in range(DPC):
                nc.sync.dma_start(
                    out=Sall[:, d * B : (d + 1) * B], in_=XTr[d, :, :]
                )
            W2sb = aux_pool.tile([4 * H, 4 * N], DT, name="W2sb")
            nc.sync.dma_start(out=W2sb[:, :], in_=W2D[:, :])
            b2sb = aux_pool.tile([4 * H, N], f32, name="b2sb")
            nc.sync.dma_start(out=b2sb[:, :], in_=B2D[:, :])

            # Two 512-chunks share one layer-1 PSUM tile / activation op.
            # Layer-2 matmuls trail by LAG pairs so the in-order PE never
            # stalls on the activation that produces its rhs.
            LAG = 2
            pend = []  # (t, c, hs_pair, op) awaiting layer-2 emission

            def emit_l2(t, c, hs, op):
                # chunk c's [4, CW] out2 sits at partition 32*(c%4); one
                # DVE copy + 4 strided DMAs then retire 4 chunks at once
                j = c % 4
                nc.tensor.matmul(
                    op[32 * j : 32 * j + DPC, :],
                    lhsT=W2sb[:, 4 * t : 4 * t + 4],
                    rhs=hs[:, (c % 2) * CW : (c % 2 + 1) * CW],
                    start=True,
                    stop=True,
                    tile_position=(0, 32 * j),
                )
                if j == 3:
                    q = c // 4
                    osb = o_pool.tile([4 * H, CW], DT, tag="osb", name="osb")
                    if b2_nonzero:
                        nc.vector.tensor_scalar_add(
                            osb[:, :], op[:, :], b2sb[:, t : t + 1]
                        )
                    else:
                        nc.vector.tensor_copy(osb[:, :], op[:, :])
                    osr = osb.rearrange("(j d) w -> j d w", j=4)
                    for d in range(DPC):
                        nc.sync.dma_start(
                            out=Sall[t : t + 1,
                                     d * B + q * 4 * CW : d * B + (q + 1) * 4 * CW],
                            in_=osr[:, d, :],
                        )

            op = None
            for t in range(N):
                wt = w_pool.tile([NP1, 4 * H], DT, tag="wt", name="wt")
                nc.sync.dma_start(out=wt[:, :], in_=WD[t, :, :])
                for pair in range(CH // 2):
                    c0 = 2 * pair
                    if c0 % 4 == 0:
                        op = po_pool.tile([4 * H, CW], f32, tag="op", name="op")
                    hp = ph_pool.tile([4 * H, 2 * CW], f32, tag="hp", name="hp")
                    for cc in range(2):
                        c = c0 + cc
                        for d in range(DPC):
                            nc.tensor.matmul(
                                hp[32 * d : 32 * (d + 1),
                                   cc * CW : (cc + 1) * CW],
                                lhsT=wt[:, 32 * d : 32 * (d + 1)],
                                rhs=Sall[:, d * B + c * CW : d * B + (c + 1) * CW],
                                start=True,
                                stop=True,
                                tile_position=(0, 32 * d),
                            )
                    hs = h_pool.tile([4 * H, 2 * CW], DT, tag="hs", name="hs")
                    if pair % 4 == 3:
                        # vector-engine leaky_relu: max(x, alpha*x)
                        tp = tmp_pool.tile([4 * H, 2 * CW], f32, tag="tp",
                                           name="tp")
                        nc.vector.tensor_scalar_mul(tp[:, :], hp[:, :],
                                                    float(ALPHA))
                        nc.vector.tensor_tensor(
                            hs[:, :], hp[:, :], tp[:, :],
                            op=mybir.AluOpType.max,
                        )
                    else:
                        nc.scalar.activation(
                            hs[:, :], hp[:, :],
                            mybir.ActivationFunctionType.Lrelu,
                            bias=0.0, scale=1.0, alpha=float(ALPHA),
                        )
                    pend.append((t, c0, hs, op))
                    if len(pend) > LAG:
                        tt, cc0, hh, oo = pend.pop(0)
                        emit_l2(tt, cc0, hh, oo)
                        emit_l2(tt, cc0 + 1, hh, oo)
            while pend:
                tt, cc0, hh, oo = pend.pop(0)
                emit_l2(tt, cc0, hh, oo)
                emit_l2(tt, cc0 + 1, hh, oo)
            nc.sync.dma_start(out=OUT[:, :], in_=Sall[:N, :])
    nc.compile()
    return nc


def kernel(x, A, order, do_idx, W1, b1, W2, b2):
    x = np.asarray(x, np.float32)
    A = np.asarray(A, np.float32)
    order = np.asarray(order)
    W1 = np.asarray(W1, np.float32)
    b1 = np.asarray(b1, np.float32)
    W2 = np.asarray(W2, np.float32)
    b2 = np.asarray(b2, np.float32)
    do = int(np.asarray(do_idx))
    np_dtype = np.float32 if os.environ.get("KERNEL_FP32") else BF16

    packs = [_pack_core(k, x, A, order, do, W1, b1, W2, b2, np_dtype)
             for k in range(NCORES)]

    if os.environ.get("KERNEL_NUMPY_EMU"):
        outs = [_emulate_core(m) for m in packs]
    else:
        key = (bool(np.any(b2 != 0.0)), np_dtype)
        if key not in _PROGRAM_CACHE:
            _PROGRAM_CACHE[key] = _build_program(key[0], np_dtype)
        nc = _PROGRAM_CACHE[key]
        from concourse.bass_utils import run_bass_kernel_spmd

        in_maps = [
            {"XT": m["XT"], "WD": m["WD"], "W2D": m["W2D"], "B2D": m["B2D"]}
            for m in packs
        ]
        res = run_bass_kernel_spmd(nc, in_maps, list(range(NCORES)))
        outs = [res.results[k]["OUT"].astype(np.float32)
                for k in range(NCORES)]

    out_full = np.empty((D, B, N), np.float32)
    for k in range(NCORES):
        O = outs[k]  # [N(topo rows), DPC*B]
        for d in range(DPC):
            dag = k * DPC + d
            od = order[dag].astype(np.int64)
            out_full[dag][:, od] = O[:, d * B : (d + 1) * B].T
    return out_full
